# revision 19
# baseline (speedup 1.0000x reference)
"""Trainium2 Bass kernel for DiagGraphSAGENet (GraphSAGE message passing).

Computes, for node features x [N, 512] and edge list [2, E]:
    agg   = segment_sum(x[src], dst)                      # sum over in-edges
    loc   = clip(agg @ Wl1.T + bl1 + x @ Wr1.T, -100, 100)
    scale = min(softplus(agg @ Wl2.T + bl2 + x @ Wr2.T) + 0.001, 100)

Strategy (8 NeuronCores, SPMD single NEFF):
  - Destination-node sharding: core c owns nodes [c*6250, (c+1)*6250).
  - Host sorts edges by (dst core, 128-node dst block, src-half); device
    gathers source rows with the Q7 dma_gather primitive (int16 indices,
    so x is split at row 32768 into lo/hi gather sources). Gather runs in
    fp16 (x pre-cast on host) to halve HBM traffic — the roofline term.
  - Segment sum realized as one-hot matmul: per 128-edge tile, build
    M[edge, dst_local] = (dstloc[edge] == iota) on DVE (one batched op
    per block), accumulate psum_agg[128 nodes, 512] += M.T @ Xe.
  - agg transposed on-PE; the four 512x512 GEMMs run in fp16 against
    host-pretransposed weights, ordered so each stationary operand is
    loaded once and streamed against both heads; bias matmuls are elided
    when biases are zero (they are for this model); clip on DVE.
  - softplus = Ln(Exp(z)+1) on ACT. Per-block results are staged into a
    contiguous group buffer (8 blocks) and Exp/Ln run as ONE wide op per
    group, so the exp<->ln activation-table reload (1.3us each) happens
    12x instead of 98x, and output DMAs coalesce to 2MB transfers.
  - Outputs are written fp16 (|loc|,|scale| <= 100, so ~0.05% rounding)
    and upcast to f32 on the host.
  - Each core writes loc/scale for its 6250 nodes; host reassembles.
"""

import math
import numpy as np

# ---------------------------------------------------------------- config

class Cfg:
    def __init__(self, n_nodes=50000, n_edges=800000, d=512, n_cores=8):
        self.N = n_nodes
        self.E = n_edges
        self.D = d
        self.C = n_cores
        assert n_nodes % n_cores == 0
        self.NPC = n_nodes // n_cores            # nodes per core
        self.B = math.ceil(self.NPC / 128)       # dst blocks per core
        self.ROWS = self.B * 128                 # padded out rows per core
        self.SPLIT = min(32768, n_nodes)         # int16 gather split point
        # max 128-row tiles per dma_gather instruction
        self.gather_chunk_tiles = 8


CFG = Cfg()

# ---------------------------------------------------------------- host prep

def _prep_edges(cfg, src, dst):
    """Sort/pad edges into per-core per-block gather streams.

    Returns (caps [B,2] shared tile caps, zfrom [B,2] first tile that may
    contain pad slots (= caps when none), total_tiles,
    idx_grid [C,16,8*total_tiles] int16, dstloc [C,128,total_tiles] f32).
    """
    C, B, NPC, SPLIT = cfg.C, cfg.B, cfg.NPC, cfg.SPLIT
    ecore = dst // NPC
    eblk = (dst % NPC) // 128
    eslot = (dst % NPC) % 128
    ehi = (src >= SPLIT).astype(np.int64)
    key = (ecore * B + eblk) * 2 + ehi
    order = np.argsort(key, kind="stable")
    src_s = src[order]
    eslot_s = eslot[order]
    counts = np.bincount(key, minlength=C * B * 2)
    start = np.concatenate([[0], np.cumsum(counts)])
    cnt = counts.reshape(C, B, 2)
    caps = -(-cnt // 128)                # ceil tiles per (core, blk, stream)
    caps = caps.max(axis=0)              # [B, 2] shared across cores (SPMD)
    tile_off = np.zeros((B, 2), np.int64)
    off = 0
    for b in range(B):
        for s in range(2):
            tile_off[b, s] = off
            off += caps[b, s]
    total_tiles = int(off)

    # pad gather slots with idx=0 (a real row, so every descriptor is
    # valid and num_idxs_reg can stay static/SPMD); padded slots carry
    # dstloc=-1 so the one-hot row is all-zero.
    zfrom = caps.copy()
    idx_grid = np.zeros((C, 16, 8 * total_tiles), np.int16)
    dstloc = np.full((C, 128, total_tiles), -1.0, np.float32)
    for c in range(C):
        for b in range(B):
            for s in (0, 1):
                T = int(caps[b, s])
                if T == 0:
                    continue
                k = (c * B + b) * 2 + s
                n = int(counts[k])
                zfrom[b, s] = min(zfrom[b, s], n // 128)
                toff = int(tile_off[b, s])
                buf = np.zeros(T * 128, np.int64)
                buf[:n] = src_s[start[k]:start[k] + n] - (SPLIT if s else 0)
                dl = np.full(T * 128, -1.0, np.float32)
                dl[:n] = eslot_s[start[k]:start[k] + n]
                idx_grid[c, :, 8 * toff:8 * (toff + T)] = (
                    buf.reshape(-1, 16).T.astype(np.int16))
                dstloc[c, :, toff:toff + T] = dl.reshape(T, 128).T
    return caps, zfrom, total_tiles, idx_grid, dstloc


def _prep_host(cfg, x, edge_index, Wl1, bl1, Wr1, Wl2, bl2, Wr2):
    src = np.asarray(edge_index[0]).astype(np.int64)
    dst = np.asarray(edge_index[1]).astype(np.int64)
    caps, zfrom, total_tiles, idx_grid, dstloc = _prep_edges(cfg, src, dst)

    xg = np.asarray(x, dtype=np.float16)

    # per-core transposed own features (fp16), padded to ROWS columns
    xt = np.zeros((cfg.C, cfg.D, cfg.ROWS), np.float16)
    for c in range(cfg.C):
        xt[c][:, :cfg.NPC] = xg[c * cfg.NPC:(c + 1) * cfg.NPC].T

    # weights packed as [128, 16*D]: for w in (Wl1, Wr1, Wl2, Wr2), chunks
    # c of W.T: rows c*128..c*128+127 -> columns (w*4+c)*D .. +D
    kc = cfg.D // 128
    packs = []
    for W in (Wl1, Wr1, Wl2, Wr2):
        WT = np.asarray(W, np.float32).T.astype(np.float16)  # [D_in, D_out]
        packs.append(WT.reshape(kc, 128, cfg.D).transpose(1, 0, 2)
                     .reshape(128, kc * cfg.D))
    wts = np.concatenate(packs, axis=1)                      # [128, 4*kc*D]

    bl1 = np.asarray(bl1, np.float32)
    bl2 = np.asarray(bl2, np.float32)
    has_bias = bool(np.any(bl1) or np.any(bl2))
    bias = np.concatenate([bl1, bl2]).astype(np.float16)[None, :]  # [1, 2D]
    iota = np.tile(np.arange(128, dtype=np.float16), (128, 1))     # [128,128]
    iden = np.eye(128, dtype=np.float16)
    ones = np.ones((1, 128), np.float16)

    in_maps = []
    for c in range(cfg.C):
        m = {
            "x": xg,
            "idx": np.tile(idx_grid[c], (8, 1)),
            "dstloc": dstloc[c],
            "xt": xt[c],
            "wts": wts,
            "iota": iota,
            "iden": iden,
        }
        if has_bias:
            m["bias"] = bias
            m["ones"] = ones
        in_maps.append(m)
    return caps, zfrom, total_tiles, has_bias, in_maps


# ---------------------------------------------------------------- device

def _build_program(cfg, caps, zfrom, total_tiles, has_bias):
    import concourse.bacc as bacc
    import concourse.mybir as mybir
    import concourse.tile as tile

    f32 = mybir.dt.float32
    f16 = mybir.dt.float16
    D, B, SPLIT, N = cfg.D, cfg.B, cfg.SPLIT, cfg.N
    kc = D // 128

    nc = bacc.Bacc("TRN2", target_bir_lowering=False, debug=False)
    x_d = nc.dram_tensor("x", [N, D], f16, kind="ExternalInput")
    idx_d = nc.dram_tensor("idx", [128, 8 * total_tiles], mybir.dt.int16,
                           kind="ExternalInput")
    dstloc_d = nc.dram_tensor("dstloc", [128, total_tiles], f32,
                              kind="ExternalInput")
    xt_d = nc.dram_tensor("xt", [D, cfg.ROWS], f16, kind="ExternalInput")
    wts_d = nc.dram_tensor("wts", [128, 4 * kc * D], f16,
                           kind="ExternalInput")
    iota_d = nc.dram_tensor("iota", [128, 128], f16, kind="ExternalInput")
    iden_d = nc.dram_tensor("iden", [128, 128], f16, kind="ExternalInput")
    if has_bias:
        bias_d = nc.dram_tensor("bias", [1, 2 * D], f16, kind="ExternalInput")
        ones_d = nc.dram_tensor("ones", [1, 128], f16, kind="ExternalInput")
    loc_d = nc.dram_tensor("loc", [cfg.ROWS, D], f16, kind="ExternalOutput")
    scale_d = nc.dram_tensor("scale", [cfg.ROWS, D], f16,
                             kind="ExternalOutput")

    Tmax = int((caps[:, 0] + caps[:, 1]).max())
    G = 8  # epilogue group size (blocks per fused Exp/Ln + output DMA)

    with tile.TileContext(nc) as tc:
        with (
            tc.tile_pool(name="const", bufs=1) as constp,
            tc.tile_pool(name="gbuf", bufs=2) as gpool,
            tc.tile_pool(name="work", bufs=3) as wpool,
            tc.tile_pool(name="grp", bufs=2) as grpool,
            tc.tile_pool(name="grp1", bufs=1) as grpool1,
            tc.tile_pool(name="mbuf", bufs=4) as mpool,
            tc.tile_pool(name="psum", bufs=2, space="PSUM") as pp,
        ):
            idx_s = constp.tile([128, 8 * total_tiles], mybir.dt.int16)
            nc.sync.dma_start(idx_s[:], idx_d[:])
            dstloc_s = constp.tile([128, total_tiles], f32)
            nc.sync.dma_start(dstloc_s[:], dstloc_d[:])
            wts_s = constp.tile([128, 4 * kc * D], f16)
            nc.sync.dma_start(wts_s[:], wts_d[:])
            iota_s = constp.tile([128, 128], f16)
            nc.sync.dma_start(iota_s[:], iota_d[:])
            ident_s = constp.tile([128, 128], f16)
            nc.sync.dma_start(ident_s[:], iden_d[:])
            if has_bias:
                bias_s = constp.tile([1, 2 * D], f16)
                nc.sync.dma_start(bias_s[:], bias_d[:])
                ones_s = constp.tile([1, 128], f16)
                nc.sync.dma_start(ones_s[:], ones_d[:])

            # One-time scrub of the gather ring buffers: pad slots skipped
            # by dma_gather (idx=-1) must never hold NaN bit patterns, since
            # the one-hot matmul multiplies them by 0 (0*NaN=NaN). After this
            # they only ever hold stale-but-finite x rows.
            for _ in range(2):
                g0 = gpool.tile([128, Tmax * D], f16, tag="gx")
                nc.vector.memset(g0[:], 0.0)
            locg = zg = None
            for b in range(B):
                gi = b % G                     # slot within epilogue group
                if gi == 0:
                    gn = min(G, B - b)         # blocks in this group
                    locg = grpool.tile([128, G, D], f16, tag="locg")
                    zg = grpool.tile([128, G, D], f32, tag="zg")
                Tlo, Thi = int(caps[b, 0]), int(caps[b, 1])
                Tb = Tlo + Thi
                toff = int(np.sum(caps[:b]))  # tiles before block b
                # ---- gather source rows for this block's edges
                if Tb > 0:
                    gx = gpool.tile([128, Tmax * D], f16, tag="gx")
                    GC = cfg.gather_chunk_tiles
                    for seg_T, seg_src, seg_t0, dst_t0 in (
                            (Tlo, x_d[0:SPLIT, :], toff, 0),
                            (Thi, x_d[SPLIT:N, :], toff + Tlo, Tlo)):
                        for t0 in range(0, seg_T, GC):
                            tn = min(GC, seg_T - t0)
                            nc.gpsimd.dma_gather(
                                out_ap=gx[:, (dst_t0 + t0) * D:
                                          (dst_t0 + t0 + tn) * D].rearrange(
                                    "p (t e) -> p t e", e=D),
                                in_ap=seg_src,
                                idxs_ap=idx_s[:, 8 * (seg_t0 + t0):
                                              8 * (seg_t0 + t0 + tn)],
                                num_idxs=tn * 128, num_idxs_reg=tn * 128,
                                elem_size=D)
                # ---- own features (transposed) for this block
                xt_s = wpool.tile([128, kc, 128], f16, tag="xt")
                nc.sync.dma_start(
                    xt_s[:],
                    xt_d[:, b * 128:(b + 1) * 128].rearrange(
                        "(c p) n -> p c n", p=128))
                # ---- aggregation: psum_agg[node, feat] += M.T @ Xe with
                # one-hot M[edge, dst] = (iota == dstloc) built per tile on
                # DVE (tensor_scalar w/ per-partition scalar ptr, 4x mode)
                agg_s = wpool.tile([128, D], f16, tag="aggs")
                if Tb > 0:
                    ps_agg = pp.tile([128, D], f32, tag="agg")
                    for t in range(Tb):
                        m = mpool.tile([128, 128], f16, tag="m")
                        nc.vector.tensor_scalar(
                            m[:], iota_s[:],
                            dstloc_s[:, toff + t:toff + t + 1], None,
                            mybir.AluOpType.is_equal)
                        nc.tensor.matmul(
                            ps_agg[:], lhsT=m[:],
                            rhs=gx[:, t * D:(t + 1) * D],
                            start=(t == 0), stop=(t == Tb - 1))
                    nc.scalar.activation(
                        agg_s[:], ps_agg[:],
                        mybir.ActivationFunctionType.Copy)
                else:
                    nc.vector.memset(agg_s[:], 0.0)
                # ---- transpose agg -> aggT (feat-major for GEMM lhsT)
                ps_t = pp.tile([128, D], f16, tag="aggT")
                for ch in range(kc):
                    nc.tensor.transpose(
                        ps_t[:, ch * 128:(ch + 1) * 128],
                        agg_s[:, ch * 128:(ch + 1) * 128],
                        ident_s[:])
                aggT_s = wpool.tile([128, D], f16, tag="aggTs")
                nc.vector.tensor_copy(aggT_s[:], ps_t[:])
                # ---- GEMMs: loc / scale heads share each stationary chunk
                ps_loc = pp.tile([128, D], f32, tag="loc")
                ps_scl = pp.tile([128, D], f32, tag="scl")
                nmm = 2 * kc
                for i in range(nmm):
                    lhsT = (aggT_s[:, (i % kc) * 128:(i % kc + 1) * 128]
                            if i < kc else xt_s[:, i - kc, :])
                    woff = (i % kc if i < kc else 1 * kc + (i - kc))
                    nc.tensor.matmul(
                        ps_loc[:], lhsT=lhsT,
                        rhs=wts_s[:, woff * D:(woff + 1) * D],
                        start=(i == 0), stop=(not has_bias and i == nmm - 1))
                    nc.tensor.matmul(
                        ps_scl[:], lhsT=lhsT,
                        rhs=wts_s[:, (2 * kc + woff) * D:
                                  (2 * kc + woff + 1) * D],
                        start=(i == 0), stop=(not has_bias and i == nmm - 1))
                if has_bias:
                    nc.tensor.matmul(
                        ps_loc[:], lhsT=ones_s[:], rhs=bias_s[:, 0:D],
                        start=False, stop=True)
                    nc.tensor.matmul(
                        ps_scl[:], lhsT=ones_s[:], rhs=bias_s[:, D:2 * D],
                        start=False, stop=True)
                # ---- per-block epilogue staging into the group buffers
                nc.vector.tensor_scalar(
                    locg[:, gi, :], ps_loc[:], -100.0, 100.0,
                    mybir.AluOpType.max, mybir.AluOpType.min)
                nc.scalar.activation(
                    zg[:, gi, :], ps_scl[:],
                    mybir.ActivationFunctionType.Copy)
                # ---- group flush: one wide Exp + Ln (softplus) + outputs.
                # Overflow of exp to inf is absorbed by the min(., 100)
                # since softplus(z) ~= z > 100 there anyway.
                if gi == gn - 1:
                    g0 = b - gi
                    eg = grpool1.tile([128, G, D], f32, tag="eg")
                    nc.scalar.activation(
                        eg[:, :gn, :], zg[:, :gn, :],
                        mybir.ActivationFunctionType.Exp)
                    spg = grpool.tile([128, G, D], f16, tag="spg")
                    nc.scalar.activation(
                        spg[:, :gn, :], eg[:, :gn, :],
                        mybir.ActivationFunctionType.Ln, bias=1.0)
                    sclg = grpool.tile([128, G, D], f16, tag="sclg")
                    nc.vector.tensor_scalar(
                        sclg[:, :gn, :], spg[:, :gn, :], 0.001, 100.0,
                        mybir.AluOpType.add, mybir.AluOpType.min)
                    nc.sync.dma_start(
                        loc_d[g0 * 128:(g0 + gn) * 128, :].rearrange(
                            "(i p) d -> p i d", p=128),
                        locg[:, :gn, :])
                    nc.sync.dma_start(
                        scale_d[g0 * 128:(g0 + gn) * 128, :].rearrange(
                            "(i p) d -> p i d", p=128),
                        sclg[:, :gn, :])

    nc.compile()
    return nc


# ---------------------------------------------------------------- driver

_CACHE = {}


def _get_program(cfg, caps, zfrom, total_tiles, has_bias):
    key = (cfg.N, cfg.E, cfg.D, cfg.C, cfg.gather_chunk_tiles, has_bias,
           caps.tobytes(), zfrom.tobytes())
    if key not in _CACHE:
        _CACHE[key] = _build_program(cfg, caps, zfrom, total_tiles, has_bias)
    return _CACHE[key]


def _run_on_hw(nc, in_maps, cfg):
    from concourse.bass_utils import run_bass_kernel_spmd
    res = run_bass_kernel_spmd(nc, in_maps, core_ids=list(range(cfg.C)))
    return res.results


def _assemble(cfg, results):
    N, D, NPC = cfg.N, cfg.D, cfg.NPC
    loc = np.empty((N, D), np.float32)
    scale = np.empty((N, D), np.float32)
    for c in range(cfg.C):
        loc[c * NPC:(c + 1) * NPC] = results[c]["loc"][:NPC]
        scale[c * NPC:(c + 1) * NPC] = results[c]["scale"][:NPC]
    return loc, scale  # fp16 device outputs upcast via assignment


def run(x, edge_index, Wl1, bl1, Wr1, Wl2, bl2, Wr2, cfg=None):
    cfg = cfg or CFG
    caps, zfrom, total_tiles, has_bias, in_maps = _prep_host(
        cfg, x, edge_index, Wl1, bl1, Wr1, Wl2, bl2, Wr2)
    nc = _get_program(cfg, caps, zfrom, total_tiles, has_bias)
    results = _run_on_hw(nc, in_maps, cfg)
    return _assemble(cfg, results)


def kernel(x, edge_index, Wl1, bl1, Wr1, Wl2, bl2, Wr2):
    return run(x, edge_index, Wl1, bl1, Wr1, Wl2, bl2, Wr2)


# ---------------------------------------------------------------- bench

def _collect_io(nc):
    import concourse.mybir as mybir
    import jax
    part_name = (nc.partition_id_tensor.name
                 if nc.partition_id_tensor else None)
    in_names, out_names, out_avals = [], [], []
    for alloc in nc.m.functions[0].allocations:
        if not isinstance(alloc, mybir.MemoryLocationSet):
            continue
        name = alloc.memorylocations[0].name
        if alloc.kind == "ExternalInput":
            if name != part_name:
                in_names.append(name)
        elif alloc.kind == "ExternalOutput":
            out_names.append(name)
            out_avals.append(jax.core.ShapedArray(
                tuple(alloc.tensor_shape), mybir.dt.np(alloc.dtype)))
    return in_names, out_names, out_avals, part_name


def _make_runner(nc, in_maps, n_cores, chain=1):
    """Build a jitted executor running the NEFF `chain` times back-to-back.

    Iteration i+1 takes iteration i's outputs as its out-buffer operands,
    which serializes the executions and defeats CSE, so (t_chainN - t_chain1)
    / (N - 1) isolates per-execution device time from dispatch/RTT overhead.
    """
    import jax
    from jax.sharding import Mesh, PartitionSpec, NamedSharding
    from jax.experimental.shard_map import shard_map
    from concourse import bass2jax

    bass2jax.install_neuronx_cc_hook()
    in_names, out_names, out_avals, part_name = _collect_io(nc)
    devices = jax.devices()[:n_cores]
    mesh = Mesh(np.asarray(devices), ("core",))
    P = PartitionSpec
    bind_names = tuple(in_names) + tuple(out_names) + (
        (part_name,) if part_name else ())
    n_in = len(in_names)

    def _body(*args):
        ins = list(args[:n_in])
        outs = list(args[n_in:])
        for _ in range(chain):
            operands = ins + outs
            if part_name:
                operands.append(bass2jax.partition_id_tensor())
            outs = list(bass2jax._bass_exec_p.bind(
                *operands,
                out_avals=tuple(out_avals),
                in_names=bind_names,
                out_names=tuple(out_names),
                lowering_input_output_aliases=(),
                sim_require_finite=True,
                sim_require_nnan=True,
                nc=nc))
        return tuple(outs)

    concat_in = [np.concatenate([m[nm] for m in in_maps], axis=0)
                 for nm in in_names]
    concat_in += [np.zeros((n_cores * a.shape[0], *a.shape[1:]), a.dtype)
                  for a in out_avals]
    dev_in = [jax.device_put(a, NamedSharding(mesh, P("core")))
              for a in concat_in]
    fn = jax.jit(shard_map(
        _body, mesh=mesh,
        in_specs=(P("core"),) * len(dev_in),
        out_specs=(P("core"),) * len(out_names),
        check_rep=False))
    return fn, dev_in


def _time_runner(fn, dev_in, reps):
    import time
    import jax
    jax.block_until_ready(fn(*dev_in))  # compile + warm
    times = []
    for _ in range(reps):
        t0 = time.perf_counter()
        jax.block_until_ready(fn(*dev_in))
        times.append(time.perf_counter() - t0)
    return min(times)


def _pipeline_time(fn, dev_in, n, trials):
    import time
    import jax
    jax.block_until_ready(fn(*dev_in))  # compile + warm
    best = float("inf")
    for _ in range(trials):
        t0 = time.perf_counter()
        outs = None
        for _ in range(n):
            outs = fn(*dev_in)
        jax.block_until_ready(outs)
        best = min(best, time.perf_counter() - t0)
    return best


def _build_trivial():
    import concourse.bacc as bacc
    import concourse.mybir as mybir
    import concourse.tile as tile
    nc = bacc.Bacc("TRN2", target_bir_lowering=False, debug=False)
    a_d = nc.dram_tensor("a", [128, 128], mybir.dt.float32,
                         kind="ExternalInput")
    b_d = nc.dram_tensor("b", [128, 128], mybir.dt.float32,
                         kind="ExternalOutput")
    with tile.TileContext(nc) as tc:
        with tc.tile_pool(name="p", bufs=1) as pool:
            t = pool.tile([128, 128], mybir.dt.float32)
            nc.sync.dma_start(t[:], a_d[:])
            nc.sync.dma_start(b_d[:], t[:])
    nc.compile()
    return nc


def bench_ns(x, edge_index, Wl1, bl1, Wr1, Wl2, bl2, Wr2,
             cfg=None, n=64, trials=10):
    """Measure per-execution device time.

    N executions are dispatched asynchronously (device queue serializes
    them; the ~80ms axon RTT is paid once), so the slope
    (t_N - t_1)/(N-1) is per-execution time including the NEFF launch
    overhead. The same slope of a trivial 2-DMA program measures that
    launch overhead alone; the difference is kernel device time.
    """
    cfg = cfg or CFG
    caps, zfrom, total_tiles, has_bias, in_maps = _prep_host(
        cfg, x, edge_index, Wl1, bl1, Wr1, Wl2, bl2, Wr2)
    nc = _get_program(cfg, caps, zfrom, total_tiles, has_bias)
    fn, dev_in = _make_runner(nc, in_maps, cfg.C, chain=1)
    if "trivial" not in _CACHE:
        _CACHE["trivial"] = _build_trivial()
    tnc = _CACHE["trivial"]
    tiny = [{"a": np.zeros((128, 128), np.float32)} for _ in range(cfg.C)]
    tfn, tdev = _make_runner(tnc, tiny, cfg.C, chain=1)

    f1 = _pipeline_time(fn, dev_in, 1, trials)
    fN = _pipeline_time(fn, dev_in, n, trials)
    t1 = _pipeline_time(tfn, tdev, 1, trials)
    tN = _pipeline_time(tfn, tdev, n, trials)
    slope_full = (fN - f1) / (n - 1)
    slope_triv = (tN - t1) / (n - 1)
    ns = (slope_full - slope_triv) * 1e9
    return ns, {"slope_full_ms": slope_full * 1e3,
                "slope_trivial_ms": slope_triv * 1e3}
